# revision 3
# baseline (speedup 1.0000x reference)
"""Cross-entropy with label smoothing on 8 TRN2 NeuronCores — bf16 stream.

Problem: inputs (B=2048, K=50257) f32 logits, targets (B,) int64.
  log_probs = log_softmax(inputs, axis=1)
  per_row = -((1-eps)*log_probs[r, t_r] + (eps/K) * sum_k log_probs[r, k])
  out = mean(per_row)   (f32 scalar)
       = mean_r [ lse_r - (1-eps)*x[r,t_r] - (eps/K)*sumx_r ]

The f32 baseline is DMA-bound at the ~358 GB/s per-NeuronCore HBM limit
(51.5 MB/core ≈ 144 us).  This version halves HBM traffic by downcasting
the logits to bf16 on the host (loss rel-err from bf16 logit rounding is
~1e-4, far under the 2e-2 gate), which shifts the bottleneck to the ACT
engine (exp at 1 elem/lane/cycle @ 1.2 GHz ≈ 84 us/core + per-op
overhead).

Sharding: batch dim across 8 cores (256 rows each).  The host pads K to
K_PAD=50304 with zeros (subtracting the pad's exp(0)=1 contribution
exactly on the host) so chunk widths stay regular.  Per core, each
(128, w) bf16 chunk is:
  ACT: exp with fused accum_out -> per-row sum(exp) strip column
  DVE: tensor_scalar(+0.0) with fused accum_out -> per-row sum(x) strip
       column (single-src op: eligible for the 4x bf16 perf mode, vs 1x
       for tensor_reduce which would be ~105 us and become the wall)
Strips are reduced to per-row scalars at the end and shipped (128x4 f32)
to the host, which does the tiny O(B) combine: lse = log(sumexp - pad),
per_row = lse - (1-eps)*x_t - (eps/K)*sumx, loss = mean.

Chunk widths are graded (small head chunk -> ACT starts early; small tail
chunk -> short drain after the last DMA lands).
"""

import numpy as np
import ml_dtypes
from contextlib import ExitStack

import concourse.bacc as bacc
import concourse.bass as bass
import concourse.mybir as mybir
import concourse.tile as tile
from concourse.bass_utils import run_bass_kernel_spmd

B = 2048
K = 50257
K_PAD = 50304                         # 2^7 * 3 * 131; zero-padded tail
N_PAD = K_PAD - K                     # 47 columns of exp(0)=1 per row
EPS = 0.1
N_CORES = 8
ROWS_PER_CORE = B // N_CORES          # 256
ROW_TILES = ROWS_PER_CORE // 128      # 2

BF16 = ml_dtypes.bfloat16

_NC_CACHE = None


def _chunk_widths(taper_head, taper_tail):
    """Per-row-tile chunk widths summing to K_PAD.  Graded ends: a small
    first chunk lets ACT start sooner after the first DMA; a small last
    chunk shortens the ACT drain after the final DMA lands."""
    head = [1572, 3144] if taper_head else []
    tail = [3144, 1572] if taper_tail else []
    mid = K_PAD - sum(head) - sum(tail)
    # middle in equal even chunks of ~6-8k
    n_mid = max(1, round(mid / 7000))
    w = mid // n_mid
    w -= w % 2
    widths = head + [w] * (n_mid - 1) + [mid - w * (n_mid - 1)] + tail
    assert sum(widths) == K_PAD and all(x % 2 == 0 for x in widths)
    return widths


def _emit_body(nc, tc, ctx, x, out, sumx_mode="ts", x_bufs=6, e_bufs=2,
               taper=True):
    f32 = mybir.dt.float32
    bf16 = mybir.dt.bfloat16
    X = mybir.AxisListType.X

    tile_widths = [
        _chunk_widths(taper and t == 0, taper and t == ROW_TILES - 1)
        for t in range(ROW_TILES)
    ]
    n_cols = [len(w) for w in tile_widths]
    total_cols = sum(n_cols)
    max_w = max(max(w) for w in tile_widths)

    xpool = ctx.enter_context(tc.tile_pool(name="x", bufs=x_bufs))
    epool = ctx.enter_context(tc.tile_pool(name="exp", bufs=e_bufs))
    spool = ctx.enter_context(tc.tile_pool(name="strips", bufs=1))
    rpool = ctx.enter_context(tc.tile_pool(name="res", bufs=1))

    se_strip = spool.tile([128, total_cols], f32, tag="se")
    sx_strip = spool.tile([128, total_cols], f32, tag="sx")
    if sumx_mode == "ts":
        dpool = ctx.enter_context(tc.tile_pool(name="dummy", bufs=1))
        dummy = dpool.tile([128, max_w], bf16, tag="dummy")

    idx = 0
    for t in range(ROW_TILES):
        k0 = 0
        for w in tile_widths[t]:
            xt = xpool.tile([128, max_w], bf16)
            nc.sync.dma_start(xt[:, :w], x[t * 128:(t + 1) * 128, k0:k0 + w])
            et = epool.tile([128, max_w], bf16)
            nc.scalar.activation(
                et[:, :w], xt[:, :w],
                mybir.ActivationFunctionType.Exp,
                accum_out=se_strip[:, idx:idx + 1],
            )
            if sumx_mode == "ts":
                # single-src DVE op (4x-eligible) with fused reduction
                nc.vector.tensor_scalar(
                    dummy[:, :w], xt[:, :w], 0.0, None,
                    mybir.AluOpType.add, mybir.AluOpType.add,
                    accum_out=sx_strip[:, idx:idx + 1],
                )
            else:
                nc.vector.reduce_sum(sx_strip[:, idx:idx + 1], xt[:, :w], axis=X)
            k0 += w
            idx += 1

    # strips -> per-row scalars: res[:,0:2] sum(exp) per row tile,
    # res[:,2:4] sum(x) per row tile
    res = rpool.tile([128, 4], f32, tag="res")
    c0 = 0
    for t in range(ROW_TILES):
        c1 = c0 + n_cols[t]
        nc.vector.reduce_sum(res[:, t:t + 1], se_strip[:, c0:c1], axis=X)
        nc.vector.reduce_sum(res[:, 2 + t:3 + t], sx_strip[:, c0:c1], axis=X)
        c0 = c1
    nc.sync.dma_start(out[:, :], res[:, :])


def _build_nc(sumx_mode="ts", x_bufs=6, e_bufs=2, taper=True, repeat=None):
    f32 = mybir.dt.float32
    bf16 = mybir.dt.bfloat16
    nc = bacc.Bacc("TRN2", target_bir_lowering=False)
    x = nc.dram_tensor("x", [ROWS_PER_CORE, K_PAD], bf16, kind="ExternalInput")
    # out[p, 0/1] = sum_exp of rows p, 128+p ; out[p, 2/3] = sum_x of same
    out = nc.dram_tensor("out", [128, 4], f32, kind="ExternalOutput")

    with tile.TileContext(nc) as tc, ExitStack() as ctx:
        if repeat is None:
            _emit_body(nc, tc, ctx, x, out, sumx_mode, x_bufs, e_bufs, taper)
        else:
            with tc.For_i(0, repeat, 1):
                with ExitStack() as inner:
                    _emit_body(nc, tc, inner, x, out, sumx_mode, x_bufs,
                               e_bufs, taper)
    nc.compile()
    return nc


def _prep_shards(inputs_f32):
    """f32 (B, K) -> list of per-core zero-padded bf16 (ROWS_PER_CORE, K_PAD)."""
    xp = np.zeros((B, K_PAD), dtype=BF16)
    xp[:, :K] = inputs_f32.astype(BF16)
    return [
        {"x": np.ascontiguousarray(xp[i * ROWS_PER_CORE:(i + 1) * ROWS_PER_CORE])}
        for i in range(N_CORES)
    ]


def kernel(inputs: np.ndarray, targets: np.ndarray) -> np.ndarray:
    global _NC_CACHE
    inputs = np.asarray(inputs, dtype=np.float32)
    targets = np.asarray(targets)
    assert inputs.shape == (B, K), inputs.shape

    if _NC_CACHE is None:
        _NC_CACHE = _build_nc()
    nc = _NC_CACHE

    res = run_bass_kernel_spmd(nc, _prep_shards(inputs), list(range(N_CORES)))

    outs = [res.results[i]["out"] for i in range(N_CORES)]  # each (128, 4)
    # rows of core i: tile0 = out[:,0]/out[:,2] (rows 0-127), tile1 = cols 1/3
    sum_exp = np.concatenate(
        [o[:, 0:2].T.reshape(-1) for o in outs]
    ).astype(np.float64) - N_PAD
    sumx = np.concatenate([o[:, 2:4].T.reshape(-1) for o in outs]).astype(np.float64)

    lse = np.log(sum_exp)
    tgt_val = inputs[np.arange(B), targets].astype(np.float64)
    per_row = lse - (1.0 - EPS) * tgt_val - (EPS / K) * sumx
    return np.float32(per_row.mean())
